# revision 18
# baseline (speedup 1.0000x reference)
"""GWPooling2D forward on 8 Trainium2 NeuronCores.

y[b, c, o0, o1] = sum_k m[c, o0, o1, k] * x[b, k]   (k = 20*20 input pixels)

The pooling map m depends only on the small `signal` parameter
(C=16, 2, 24, 24); it is computed on host (FFTs + 16 complex 576x576
matrix exponentials) exactly as in the reference, replicated to every
core. The heavy einsum (8192 x 400 x 4096) is data-parallel across the
8 cores: each core gets a 1024-batch shard of x (pre-transposed so the
contraction dim lies on SBUF partitions) and computes its (1024, 4096)
output slab with float32r matmuls accumulating in PSUM.
"""

import numpy as np
import scipy.linalg

import concourse.bass as bass
import concourse.bacc as bacc
import concourse.mybir as mybir
import concourse.tile as tile
from concourse.bass_utils import run_bass_kernel_spmd

C = 16
P = (24, 24)
NI = (20, 20)
NO = (16, 16)
B = 8192
NCORES = 8
BS = B // NCORES              # 1024 batch rows per core
K = NI[0] * NI[1]             # 400 contraction
O = C * NO[0] * NO[1]         # 4096 output features
KP = 100                      # contraction rows per chunk (on SBUF partitions)
KC = 4                        # chunks: KP * KC == K
BT = 128                      # batch tile (PSUM partitions)
OT = 512                      # output-feature tile (PSUM free dim)


# ---------------------------------------------------------------- host map ---

def _hann(n):
    return 0.5 * (1.0 - np.cos(2.0 * np.pi * np.arange(n) / n))


def _signal_to_spectrum(signal):
    n0, n1 = signal.shape[-2], signal.shape[-1]
    window = _hann(n0)[:, None] * _hann(n1)[None, :]
    rx = np.arange((-n0) // 2 + 1, n0 // 2 + 1)[:, None]
    ry = np.arange((-n1) // 2 + 1, n1 // 2 + 1)[None, :]
    r = (1 + rx * rx + ry * ry).astype(np.float64)
    wf = np.roll(np.fft.fft2(signal), (n0 // 2, n1 // 2), (-2, -1)) / r / 5.0
    wt = np.fft.ifft2(np.roll(wf, (-(n0 // 2), -(n1 // 2)), (-2, -1))) * window
    return np.roll(np.fft.fft2(wt), (n0 // 2, n1 // 2), (-2, -1))


def _gw2d_algebra(w):
    p0, p1 = w.shape[-2], w.shape[-1]
    pad = [(0, 0)] * (w.ndim - 2) + [(p1 // 2, p1 // 2), (p0 // 2, p0 // 2)]
    wp = np.pad(w, pad)
    ia = np.arange(p0)[:, None] + np.arange(p0)[None, :]
    jb = np.arange(p1)[:, None] + np.arange(p1)[None, :]
    ws = wp[..., ia[:, None, :, None], jb[None, :, None, :]]
    ws = ws[..., ::-1, ::-1, :, :]
    kx = np.arange((-p0) // 2 + 1, p0 // 2 + 1)[:, None]
    ky = np.arange((-p1) // 2 + 1, p1 // 2 + 1)[None, :]
    return -1j * (ws[..., 0, :, :, :, :] * kx + ws[..., 1, :, :, :, :] * ky)


def _transform_to_map(t):
    p0, p1 = t.shape[-2], t.shape[-1]
    di = (p0 - NI[0], p1 - NI[1])
    do = (p0 - NO[0], p1 - NO[1])
    x = t[..., do[0] // 2 + 1:(-do[0]) // 2 + 1, do[1] // 2 + 1:(-do[1]) // 2 + 1,
          di[0] // 2 + 1:(-di[0]) // 2 + 1, di[1] // 2 + 1:(-di[1]) // 2 + 1]
    x = np.roll(x, (NO[0] // 2 + 1, NO[1] // 2 + 1, NI[0] // 2 + 1, NI[1] // 2 + 1),
                (-4, -3, -2, -1))
    return np.fft.fft2(np.fft.ifft2(x, axes=(-2, -1)), axes=(-4, -3)).real


def compute_mf(signal):
    """signal (C,2,24,24) -> pooling matrix (O=4096, K=400) float32."""
    spectrum = _signal_to_spectrum(signal.astype(np.float64))
    p0, p1 = spectrum.shape[-2], spectrum.shape[-1]
    a = _gw2d_algebra(spectrum)
    n = p0 * p1
    mat = a.reshape(a.shape[:-4] + (n, n))
    t = np.stack([scipy.linalg.expm(mat[i]) for i in range(mat.shape[0])])
    t = t.reshape(t.shape[:-2] + (p0, p1, p0, p1))
    m = _transform_to_map(t)
    return m.reshape(O, K).astype(np.float32)


# ------------------------------------------------------------ device kernel ---

_built = None


def _build():
    global _built
    if _built is not None:
        return _built
    nc = bacc.Bacc(dynamic_dma_scratch_size=256)
    f32 = mybir.dt.float32
    f32r = mybir.dt.float32r

    xT_d = nc.declare_dram_parameter("xT", (K, BS), f32r, isOutput=False)
    mfT_d = nc.declare_dram_parameter("mfT", (K, O), f32r, isOutput=False)
    out_d = nc.declare_dram_parameter("out", (BS, O), f32, isOutput=True)

    with tile.TileContext(nc) as tc:
        with tc.tile_pool(name="xpool", bufs=1) as xpool, \
             tc.tile_pool(name="wpool", bufs=1) as wpool, \
             tc.tile_pool(name="opool", bufs=3) as opool, \
             tc.tile_pool(name="ppool", bufs=8, space="PSUM") as ppool:
            # x shard first (small); mf arrives per-co-tile so matmuls start
            # after ~4MB instead of the full 8.2MB of loads.
            xt = xpool.tile([KP, KC, BS], f32r, name="xt")
            nc.sync.dma_start(xt[:], xT_d.rearrange("(c p) b -> p c b", p=KP))
            mts = []
            for co in range(O // OT):
                mt = wpool.tile([KP, KC, OT], f32r, tag=f"mt{co}", name=f"mt{co}")
                nc.sync.dma_start(
                    mt[:],
                    mfT_d[:, co * OT:(co + 1) * OT].rearrange(
                        "(c p) o -> p c o", p=KP))
                mts.append(mt)

            # G co-tiles share one output staging tile -> 1MB stores (fewer
            # SWDGE setups); copies alternate DVE/ACT so neither engine gates
            # PSUM-slot reuse.
            G = 4
            for cp in range(O // OT // G):
                for b in range(BS // BT):
                    ot = opool.tile([BT, G * OT], f32, name="ot")
                    for j in range(G):
                        co = cp * G + j
                        ps = ppool.tile([BT, OT], f32, name="ps")
                        for ci in range(KC):
                            nc.tensor.matmul(
                                ps[:],
                                xt[:, ci, b * BT:(b + 1) * BT],
                                mts[co][:, ci, :],
                                start=(ci == 0),
                                stop=(ci == KC - 1),
                            )
                        if j % 2 == 0:
                            nc.vector.tensor_copy(ot[:, j * OT:(j + 1) * OT], ps[:])
                        else:
                            nc.scalar.copy(ot[:, j * OT:(j + 1) * OT], ps[:])
                    nc.sync.dma_start(
                        out_d[b * BT:(b + 1) * BT, cp * G * OT:(cp + 1) * G * OT],
                        ot[:])
    nc.compile()
    _built = nc
    return nc


def _run(x, signal, **spmd_kwargs):
    nc = _build()
    mf = compute_mf(np.asarray(signal))
    mfT = np.ascontiguousarray(mf.T)                       # (400, 4096)
    xT = np.asarray(x).reshape(B, K).T                     # (400, 8192)
    in_maps = [
        {"xT": np.ascontiguousarray(xT[:, i * BS:(i + 1) * BS]), "mfT": mfT}
        for i in range(NCORES)
    ]
    res = run_bass_kernel_spmd(nc, in_maps, list(range(NCORES)), **spmd_kwargs)
    y = np.concatenate([r["out"] for r in res.results], axis=0)   # (8192, 4096)
    return y.reshape(B, C, NO[0], NO[1]), res


def kernel(x, signal):
    y, _ = _run(x, signal)
    return y
